# revision 22
# baseline (speedup 1.0000x reference)
"""Trainium2 Bass kernel for nn_DecoderRNN (LSTM decoder, H=2048, T=120, B=256).

Strategy: tensor-parallel over the 4H gate dimension across 8 NeuronCores.
 - Each core owns 256 hidden units (1024 of the 8192 gate rows, permuted so the
   core holds the i/f/g/o rows of ITS hidden units). W_hh^T chunk stays
   SBUF-resident ([16 k-tiles, 128, 1024] fp32).
 - Everything runs in "transposed" layout [hidden-on-partitions, batch-on-free]
   so the recurrence needs no transposes: gatesT[1024,256] = W^T-chunkT.T @ hT.
 - x_proj (= frame @ W_ih^T + b_ih + b_hh, constant across steps) is computed
   on the host and added on-device with the vector engine.
 - Per step the new local h chunk [256,256] is exchanged with ONE AllGather
   (measured: each collective op costs ~20us of control latency regardless of
   payload size or group count and consecutive ops don't pipeline, so one big
   AG beats two pipelined half-AGs). The gather-in DMA is split in four so the
   next step's matmuls start on the first k-slots early.
 - The final FC (h @ W_fc^T) is computed as per-core partials over the local
   hidden slice; partials are summed on the host (no extra collective).
 - Compute dtype bf16 (KERNEL_MMDT=f32r gives ~2e-5 rel err at ~+25% step
   time; bf16 gives ~3e-4 — both far inside the 2e-2 gate).
"""

import os
import sys

import numpy as np

sys.path.insert(0, "/opt/trn_rl_repo")

import concourse.bass as bass  # noqa: E402
import concourse.mybir as mybir  # noqa: E402
import concourse.tile as tile  # noqa: E402
from concourse import bacc  # noqa: E402
from concourse import bass_utils  # noqa: E402

# Persist compiled executables across processes so repeated kernel() calls in
# fresh interpreters skip the multi-minute neuronxcc compile when possible.
try:
    import jax

    jax.config.update("jax_compilation_cache_dir", "/tmp/jax_cache_decoder_rnn")
    jax.config.update("jax_persistent_cache_min_compile_time_secs", 1.0)
except Exception:
    pass

H = 2048
OUTD = 66
NCLS = 10
LENGTH = 120
B = 256
IND = 76
NCORES = 8
HL = H // NCORES  # 256 hidden units per core
DT = mybir.dt.float32
F32 = mybir.dt.float32
AF = mybir.ActivationFunctionType

# PE fp32 matmul costs 4 cycles/row; float32r (same bits, reduced-precision
# accumulate) and bf16 run at 1 cycle/row (fp32r needs moving free dim >=256).
# bf16 additionally halves the bytes of the per-step h exchange (AllGather +
# DRAM hops), worth a few us/step; accuracy ~4e-3 vs fp32r's ~2e-5 — both well
# inside the 2e-2 gate. KERNEL_MMDT: bf16 (default) | f32r | f32.
_MMDT = os.environ.get("KERNEL_MMDT", "bf16")
DTR = {"bf16": mybir.dt.bfloat16, "f32r": mybir.dt.float32r, "f32": mybir.dt.float32}[_MMDT]


def _build_program(T: int, variant: str = "full"):
    # variant: "full" | "nocc" (drop collectives) | "nodma" (drop collectives +
    # h exchange DMAs) | "nomm" (drop the gate matmuls). Non-"full" variants
    # produce WRONG results and exist only for timing attribution.
    nc = bacc.Bacc(trn_type="TRN2", num_devices=NCORES, debug=False)

    w_ext = nc.declare_dram_parameter("w", [16, 128, 1024], DTR, isOutput=False)
    xp_ext = nc.declare_dram_parameter("xp", [8, 128, B], DT, isOutput=False)
    wfc_ext = nc.declare_dram_parameter("wfc", [2, 128, OUTD], DTR, isOutput=False)
    out_ext = nc.declare_dram_parameter("outp", [T, OUTD, B], F32, isOutput=True)

    with tile.TileContext(nc) as tc:
        with (
            tc.tile_pool(name="const", bufs=1) as constp,
            tc.tile_pool(name="work", bufs=3) as work,
            tc.tile_pool(name="hrhs", bufs=2) as hrhsp,
            tc.tile_pool(name="psum", bufs=1, space="PSUM") as psump,
            tc.tile_pool(name="psfcp", bufs=2, space="PSUM") as psfcp,
            tc.tile_pool(name="dram", bufs=2, space="DRAM") as dramp,
        ):
            w_sb = constp.tile([128, 16, 1024], DTR, name="w_sb")
            nc.sync.dma_start(w_sb[:], w_ext.ap().rearrange("s p m -> p s m"))
            xp_sb = constp.tile([128, 8, B], DT, name="xp_sb")
            nc.sync.dma_start(xp_sb[:], xp_ext.ap().rearrange("m p n -> p m n"))
            wfc_sb = constp.tile([128, 2, OUTD], DTR, name="wfc_sb")
            nc.sync.dma_start(wfc_sb[:], wfc_ext.ap().rearrange("s p m -> p s m"))
            c_sb = constp.tile([128, 2, B], F32, name="c_sb")

            h_rhs_prev = None
            for t in range(T):
                # Two m-tiles share one PSUM bank. Pair (2i, 2i+1) so the x=0
                # gate reads (m0-m3, banks 0-1) never share a bank with the
                # still-accumulating x=1 matmul writes (m4-m7, banks 2-3) —
                # the PSUM bank-overlap tracker serializes same-bank PE-writes
                # vs reads, which would delay h_a and the first AllGather.
                pbanks = [
                    psump.tile([128, 2 * B], F32, tag=f"pb{i}", name=f"pb{i}_{t}")
                    for i in range(4)
                ]
                psums = [pbanks[m // 2][:, (m % 2) * B : (m % 2 + 1) * B] for m in range(8)]
                if t > 0 and variant == "nomm":
                    # keep one matmul per bank so psums are written at all
                    for m in (0, 2, 4, 6):
                        nc.tensor.matmul(
                            psums[m], w_sb[:, 0, m * 128 : (m + 1) * 128],
                            h_rhs_prev[:, 0, :], start=True, stop=True,
                            skip_group_check=True,
                        )
                if t > 0 and variant != "nomm":
                    # start=True clears has_written for the ENTIRE bank, so only
                    # the first group per bank (even m) may use it; its bank-mate
                    # (m+1) relies on the cleared bits (first write = overwrite).
                    # Requires m.s0 to be issued before (m+1).s0 — phase A order
                    # m0..m7 guarantees that.
                    # k-slot s holds units [256*(s//2) + 128*(s%2), +128):
                    # the single per-step AllGather output in rank-major order.
                    for m in range(8):
                        for s in range(16):
                            nc.tensor.matmul(
                                psums[m],
                                w_sb[:, s, m * 128 : (m + 1) * 128],
                                h_rhs_prev[:, s, :],
                                start=(s == 0 and m % 2 == 0),
                                stop=(s == 15),
                                skip_group_check=True,
                            )

                h_rhs = (
                    hrhsp.tile([128, 16, B], DTR, tag="hrhs", name=f"hrhs_{t}")
                    if t < T - 1
                    else None
                )
                agin = (
                    dramp.tile([2 * 128, B], DTR, tag="agin", name=f"agin_{t}")
                    if t < T - 1 and variant != "nodma"
                    else None
                )
                psfc = psfcp.tile([OUTD, B], F32, tag="psfc", name=f"psfc_{t}")
                for x in range(2):
                    pre = {}
                    for q, (fn, nm) in enumerate(
                        [(AF.Sigmoid, "i"), (AF.Sigmoid, "f"), (AF.Tanh, "g"), (AF.Sigmoid, "o")]
                    ):
                        m = 4 * x + q
                        g_t = work.tile(
                            [128, B], F32, tag=f"g{nm}", name=f"g{nm}_{t}_{x}"
                        )
                        if t == 0:
                            nc.scalar.activation(g_t[:], xp_sb[:, m, :], fn)
                        else:
                            nc.vector.tensor_add(
                                out=g_t[:], in0=psums[m][:], in1=xp_sb[:, m, :]
                            )
                            nc.scalar.activation(g_t[:], g_t[:], fn)
                        pre[nm] = g_t

                    ig = work.tile([128, B], F32, tag="ig", name=f"ig_{t}_{x}")
                    nc.vector.tensor_mul(out=ig[:], in0=pre["i"][:], in1=pre["g"][:])
                    if t == 0:
                        nc.vector.tensor_copy(out=c_sb[:, x, :], in_=ig[:])
                    else:
                        fc_ = work.tile([128, B], F32, tag="fc", name=f"fc_{t}_{x}")
                        nc.vector.tensor_mul(
                            out=fc_[:], in0=pre["f"][:], in1=c_sb[:, x, :]
                        )
                        nc.vector.tensor_add(
                            out=c_sb[:, x, :], in0=ig[:], in1=fc_[:]
                        )
                    tc_t = work.tile([128, B], F32, tag="tc", name=f"tc_{t}_{x}")
                    nc.scalar.activation(tc_t[:], c_sb[:, x, :], AF.Tanh)
                    h_t = work.tile([128, B], DTR, tag=f"h{x}", name=f"h_{t}_{x}")
                    nc.vector.tensor_mul(out=h_t[:], in0=pre["o"][:], in1=tc_t[:])

                    # FC partial: outT[66, B] += wfc_x.T @ h_x
                    nc.tensor.matmul(
                        psfc,
                        wfc_sb[:, x, :],
                        h_t[:],
                        start=(x == 0),
                        stop=(x == 1),
                    )

                    if agin is not None:
                        # stage on the gpsimd queue: the collective trigger is
                        # next on the same queue, skipping a cross-queue sem hop
                        nc.gpsimd.dma_start(agin[x * 128 : (x + 1) * 128, :], h_t[:])

                if agin is not None:
                    agout = dramp.tile(
                        [NCORES * 256, B],
                        DTR,
                        tag="agout",
                        name=f"agout_{t}",
                        addr_space="Shared",
                    )
                    if variant == "full":
                        nc.gpsimd.collective_compute(
                            "AllGather",
                            mybir.AluOpType.bypass,
                            replica_groups=[list(range(NCORES))],
                            ins=[agin[:].opt()],
                            outs=[agout[:].opt()],
                            unique_tensors=os.environ.get("KERNEL_UT", "No"),
                        )
                    # split the gather-in so the first k-slots arrive (and the
                    # next step's matmuls can start) before the whole payload
                    # lands; leading chunks are smaller because the PE consumes
                    # slots in order and restarts on slot 0
                    ag_v = agout.rearrange("(s p) n -> p s n", p=128)
                    for lo, hi in ((0, 2), (2, 4), (4, 8), (8, 12), (12, 16)):
                        nc.sync.dma_start(
                            h_rhs[:, lo:hi, :],
                            ag_v[:, lo:hi, :],
                        )
                fc_stage = work.tile([OUTD, B], F32, tag="fcs", name=f"fcs_{t}")
                nc.scalar.copy(fc_stage[:], psfc[:])
                nc.sync.dma_start(out_ext[t], fc_stage[:])
                h_rhs_prev = h_rhs
    nc.finalize()
    return nc


def _prepare_inputs(inputs, labels, W_ih, W_hh, b_ih, b_hh, W_fc, b_fc):
    """Build per-core input maps. Returns (in_maps, frame0)."""
    inputs = np.asarray(inputs, dtype=np.float32)
    labels = np.asarray(labels)
    W_ih = np.asarray(W_ih, dtype=np.float32)
    W_hh = np.asarray(W_hh, dtype=np.float32)
    b_ih = np.asarray(b_ih, dtype=np.float32)
    b_hh = np.asarray(b_hh, dtype=np.float32)
    W_fc = np.asarray(W_fc, dtype=np.float32)
    b_fc = np.asarray(b_fc, dtype=np.float32)

    b = inputs.shape[0]
    frame0 = inputs.reshape(b, OUTD)
    enc = np.zeros((b, NCLS), dtype=np.float32)
    enc[:, int(labels[0])] = 1.0
    frame = np.concatenate([frame0, enc], axis=1)  # [B, 76]

    bias = b_ih + b_hh
    xproj = frame @ W_ih.T + bias  # [B, 8192]

    # global k-slot unit ordering: slot s<8: units 256*l + p (l=s); s>=8: 256*l+128+p
    in_maps = []
    for j in range(NCORES):
        rows = []
        for x in range(2):
            for q in range(4):
                base = q * H + HL * j + 128 * x
                rows.extend(range(base, base + 128))
        rows = np.array(rows)  # 1024 per-core gate rows

        Wj = W_hh[rows, :]  # [1024, 2048]
        # w[s, p, m] = Wj[m, unit(s,p)]; slot s = rank-major AG layout:
        # units [256*(s//2) + 128*(s%2), +128)
        w = np.empty((16, 128, 1024), dtype=np.float32)
        for s in range(16):
            l, x = s // 2, s % 2
            u0 = HL * l + 128 * x
            w[s] = Wj[:, u0 : u0 + 128].T
        xp = xproj[:, rows].T.reshape(8, 128, b).astype(np.float32)
        wfc = np.empty((2, 128, OUTD), dtype=np.float32)
        for x in range(2):
            u0 = HL * j + 128 * x
            wfc[x] = W_fc[:, u0 : u0 + 128].T
        if _MMDT == "bf16":
            import ml_dtypes

            w = w.astype(ml_dtypes.bfloat16)
            wfc = wfc.astype(ml_dtypes.bfloat16)
        in_maps.append({"w": w, "xp": np.ascontiguousarray(xp), "wfc": wfc})
    return in_maps, frame0, b_fc


_PROGRAM_CACHE = {}


def _get_program(T):
    variant = os.environ.get("KERNEL_VARIANT", "full")
    key = (T, variant)
    if key not in _PROGRAM_CACHE:
        _PROGRAM_CACHE[key] = _build_program(T, variant)
    return _PROGRAM_CACHE[key]


_RUNNER_CACHE = {}


def make_runner(nc):
    """Jitted runner for an arbitrary finalized program (mimics
    bass2jax.run_bass_via_pjrt's multi-core branch, but the traced callable is
    constructed ONCE so repeated calls hit jax's C++ fast path —
    run_bass_via_pjrt builds a fresh closure per call, which re-traces and
    re-serializes the whole BIR each time)."""
    import jax
    from jax.experimental.shard_map import shard_map
    from jax.sharding import Mesh, PartitionSpec

    from concourse import bass2jax, mybir as _mybir

    bass2jax.install_neuronx_cc_hook()
    partition_name = nc.partition_id_tensor.name if nc.partition_id_tensor else None
    in_names, out_names, out_avals, zero_outs = [], [], [], []
    for alloc in nc.m.functions[0].allocations:
        if not isinstance(alloc, _mybir.MemoryLocationSet):
            continue
        name = alloc.memorylocations[0].name
        if alloc.kind == "ExternalInput":
            if name != partition_name:
                in_names.append(name)
        elif alloc.kind == "ExternalOutput":
            shape = tuple(alloc.tensor_shape)
            dtype = _mybir.dt.np(alloc.dtype)
            out_names.append(name)
            out_avals.append(jax.core.ShapedArray(shape, dtype))
            zero_outs.append(np.zeros(shape, dtype))
    n_params = len(in_names)
    all_in_names = in_names + out_names
    if partition_name is not None:
        all_in_names.append(partition_name)

    def _body(*args):
        operands = list(args)
        if partition_name is not None:
            operands.append(bass2jax.partition_id_tensor())
        outs = bass2jax._bass_exec_p.bind(
            *operands,
            out_avals=tuple(out_avals),
            in_names=tuple(all_in_names),
            out_names=tuple(out_names),
            lowering_input_output_aliases=(),
            sim_require_finite=True,
            sim_require_nnan=True,
            nc=nc,
        )
        return tuple(outs)

    devices = jax.devices()[:NCORES]
    mesh = Mesh(np.asarray(devices), ("core",))
    n_outs = len(out_names)
    sharded = jax.jit(
        shard_map(
            _body,
            mesh=mesh,
            in_specs=(PartitionSpec("core"),) * (n_params + n_outs),
            out_specs=(PartitionSpec("core"),) * n_outs,
            check_rep=False,
        ),
        keep_unused=True,
    )
    meta = (in_names, out_names, out_avals, zero_outs, mesh)
    return (sharded, meta)


def _get_runner(T):
    key = (T, os.environ.get("KERNEL_VARIANT", "full"))
    if key not in _RUNNER_CACHE:
        _RUNNER_CACHE[key] = make_runner(_get_program(T))
    return _RUNNER_CACHE[key]


def run_fast(in_maps, T):
    """Run via the cached runner; returns list of per-core result dicts."""
    import jax

    sharded, (in_names, out_names, out_avals, zero_outs, mesh) = _get_runner(T)
    concat_in = [
        np.concatenate([in_maps[c][name] for c in range(NCORES)], axis=0)
        for name in in_names
    ]
    concat_zeros = [
        np.zeros((NCORES * z.shape[0], *z.shape[1:]), z.dtype) for z in zero_outs
    ]
    out_arrs = sharded(*concat_in, *concat_zeros)
    return [
        {
            name: np.asarray(out_arrs[i]).reshape(NCORES, *out_avals[i].shape)[c]
            for i, name in enumerate(out_names)
        }
        for c in range(NCORES)
    ]


def time_exec(in_maps, T, reps=8):
    """Time device execution with device-resident inputs (min over reps)."""
    import time as _time

    import jax

    sharded, (in_names, out_names, out_avals, zero_outs, mesh) = _get_runner(T)
    from jax.sharding import NamedSharding, PartitionSpec

    sh = NamedSharding(mesh, PartitionSpec("core"))
    dev_in = [
        jax.device_put(
            np.concatenate([in_maps[c][name] for c in range(NCORES)], axis=0), sh
        )
        for name in in_names
    ]
    dev_zero = [
        jax.device_put(np.zeros((NCORES * z.shape[0], *z.shape[1:]), z.dtype), sh)
        for z in zero_outs
    ]
    # warmup
    jax.block_until_ready(sharded(*dev_in, *dev_zero))
    best = float("inf")
    for _ in range(reps):
        t0 = _time.perf_counter()
        jax.block_until_ready(sharded(*dev_in, *dev_zero))
        best = min(best, _time.perf_counter() - t0)
    return best


def run(inputs_dict, T=LENGTH, trace=False):
    """Run the kernel; returns (full_output, exec_time_ns_or_None)."""
    in_maps, frame0, b_fc = _prepare_inputs(**inputs_dict)
    nc = _get_program(T)
    res = bass_utils.run_bass_kernel_spmd(
        nc, in_maps, core_ids=list(range(NCORES)), trace=trace
    )
    total = np.zeros((T, OUTD, B), dtype=np.float32)
    for r in res.results:
        total += r["outp"]
    total += b_fc[None, :, None]
    outs = total.transpose(2, 0, 1)  # [B, T, 66]
    full = np.concatenate([frame0[:, None, :], outs], axis=1)  # [B, T+1, 66]
    out = full.reshape(B, T + 1, 22, 3).astype(np.float32)
    return out, res.exec_time_ns


def kernel(**inputs):
    out, _ = run(inputs, T=LENGTH, trace=False)
    return out


# revision 23
# speedup vs baseline: 1.1152x; 1.1152x over previous
"""Trainium2 Bass kernel for nn_DecoderRNN (LSTM decoder, H=2048, T=120, B=256).

Strategy: tensor-parallel over the 4H gate dimension across 8 NeuronCores.
 - Each core owns 256 hidden units (1024 of the 8192 gate rows, permuted so the
   core holds the i/f/g/o rows of ITS hidden units). W_hh^T chunk stays
   SBUF-resident ([16 k-tiles, 128, 1024] fp32).
 - Everything runs in "transposed" layout [hidden-on-partitions, batch-on-free]
   so the recurrence needs no transposes: gatesT[1024,256] = W^T-chunkT.T @ hT.
 - x_proj (= frame @ W_ih^T + b_ih + b_hh, constant across steps) is computed
   on the host and added on-device with the vector engine.
 - Per step the new local h chunk [256,256] is exchanged with ONE AllGather
   (measured: each collective op costs ~20us of control latency regardless of
   payload size or group count and consecutive ops don't pipeline, so one big
   AG beats two pipelined half-AGs). The gather-in DMA is split in four so the
   next step's matmuls start on the first k-slots early.
 - The final FC (h @ W_fc^T) is computed as per-core partials over the local
   hidden slice; partials are summed on the host (no extra collective).
 - Compute dtype bf16 (KERNEL_MMDT=f32r gives ~2e-5 rel err at ~+25% step
   time; bf16 gives ~3e-4 — both far inside the 2e-2 gate).
"""

import os
import sys

import numpy as np

sys.path.insert(0, "/opt/trn_rl_repo")

import concourse.bass as bass  # noqa: E402
import concourse.mybir as mybir  # noqa: E402
import concourse.tile as tile  # noqa: E402
from concourse import bacc  # noqa: E402
from concourse import bass_utils  # noqa: E402

# Persist compiled executables across processes so repeated kernel() calls in
# fresh interpreters skip the multi-minute neuronxcc compile when possible.
try:
    import jax

    jax.config.update("jax_compilation_cache_dir", "/tmp/jax_cache_decoder_rnn")
    jax.config.update("jax_persistent_cache_min_compile_time_secs", 1.0)
except Exception:
    pass

H = 2048
OUTD = 66
NCLS = 10
LENGTH = 120
B = 256
IND = 76
NCORES = 8
HL = H // NCORES  # 256 hidden units per core
DT = mybir.dt.float32
F32 = mybir.dt.float32
AF = mybir.ActivationFunctionType

# PE fp32 matmul costs 4 cycles/row; float32r (same bits, reduced-precision
# accumulate) and bf16 run at 1 cycle/row (fp32r needs moving free dim >=256).
# bf16 additionally halves the bytes of the per-step h exchange (AllGather +
# DRAM hops), worth a few us/step; accuracy ~4e-3 vs fp32r's ~2e-5 — both well
# inside the 2e-2 gate. KERNEL_MMDT: bf16 (default) | f32r | f32.
_MMDT = os.environ.get("KERNEL_MMDT", "bf16")
DTR = {"bf16": mybir.dt.bfloat16, "f32r": mybir.dt.float32r, "f32": mybir.dt.float32}[_MMDT]


def _build_program(T: int, variant: str = "full"):
    # variant: "full" | "nocc" (drop collectives) | "nodma" (drop collectives +
    # h exchange DMAs) | "nomm" (drop the gate matmuls). Non-"full" variants
    # produce WRONG results and exist only for timing attribution.
    nc = bacc.Bacc(trn_type="TRN2", num_devices=NCORES, debug=False)

    w_ext = nc.declare_dram_parameter("w", [16, 128, 1024], DTR, isOutput=False)
    xp_ext = nc.declare_dram_parameter("xp", [8, 128, B], DT, isOutput=False)
    wfc_ext = nc.declare_dram_parameter("wfc", [2, 128, OUTD], DTR, isOutput=False)
    out_ext = nc.declare_dram_parameter("outp", [T, OUTD, B], F32, isOutput=True)

    with tile.TileContext(nc) as tc:
        with (
            tc.tile_pool(name="const", bufs=1) as constp,
            tc.tile_pool(name="work", bufs=3) as work,
            tc.tile_pool(name="hrhs", bufs=2) as hrhsp,
            tc.tile_pool(name="psum", bufs=1, space="PSUM") as psump,
            tc.tile_pool(name="psfcp", bufs=2, space="PSUM") as psfcp,
            tc.tile_pool(name="dram", bufs=2, space="DRAM") as dramp,
        ):
            w_sb = constp.tile([128, 16, 1024], DTR, name="w_sb")
            nc.sync.dma_start(w_sb[:], w_ext.ap().rearrange("s p m -> p s m"))
            xp_sb = constp.tile([128, 8, B], DT, name="xp_sb")
            nc.sync.dma_start(xp_sb[:], xp_ext.ap().rearrange("m p n -> p m n"))
            wfc_sb = constp.tile([128, 2, OUTD], DTR, name="wfc_sb")
            nc.sync.dma_start(wfc_sb[:], wfc_ext.ap().rearrange("s p m -> p s m"))
            c_sb = constp.tile([128, 2, B], F32, name="c_sb")

            h_rhs_prev = None
            for t in range(T):
                # Two m-tiles share one PSUM bank. Pair (2i, 2i+1) so the x=0
                # gate reads (m0-m3, banks 0-1) never share a bank with the
                # still-accumulating x=1 matmul writes (m4-m7, banks 2-3) —
                # the PSUM bank-overlap tracker serializes same-bank PE-writes
                # vs reads, which would delay h_a and the first AllGather.
                pbanks = [
                    psump.tile([128, 2 * B], F32, tag=f"pb{i}", name=f"pb{i}_{t}")
                    for i in range(4)
                ]
                psums = [pbanks[m // 2][:, (m % 2) * B : (m % 2 + 1) * B] for m in range(8)]
                if t > 0 and variant == "nomm":
                    # keep one matmul per bank so psums are written at all
                    for m in (0, 2, 4, 6):
                        nc.tensor.matmul(
                            psums[m], w_sb[:, 0, m * 128 : (m + 1) * 128],
                            h_rhs_prev[:, 0, :], start=True, stop=True,
                            skip_group_check=True,
                        )
                if t > 0 and variant != "nomm":
                    # start=True clears has_written for the ENTIRE bank, so only
                    # the first group per bank (even m) may use it; its bank-mate
                    # (m+1) relies on the cleared bits (first write = overwrite).
                    # Requires m.s0 to be issued before (m+1).s0 — phase A order
                    # m0..m7 guarantees that.
                    # k-slot s holds units [256*(s//2) + 128*(s%2), +128):
                    # the single per-step AllGather output in rank-major order.
                    for m in range(8):
                        for s in range(16):
                            nc.tensor.matmul(
                                psums[m],
                                w_sb[:, s, m * 128 : (m + 1) * 128],
                                h_rhs_prev[:, s, :],
                                start=(s == 0 and m % 2 == 0),
                                stop=(s == 15),
                                skip_group_check=True,
                            )

                h_rhs = (
                    hrhsp.tile([128, 16, B], DTR, tag="hrhs", name=f"hrhs_{t}")
                    if t < T - 1
                    else None
                )
                agin = (
                    dramp.tile([2 * 128, B], DTR, tag="agin", name=f"agin_{t}")
                    if t < T - 1 and variant != "nodma"
                    else None
                )
                psfc = psfcp.tile([OUTD, B], F32, tag="psfc", name=f"psfc_{t}")
                for x in range(2):
                    pre = {}
                    for q, (fn, nm) in enumerate(
                        [(AF.Sigmoid, "i"), (AF.Sigmoid, "f"), (AF.Tanh, "g"), (AF.Sigmoid, "o")]
                    ):
                        m = 4 * x + q
                        g_t = work.tile(
                            [128, B], F32, tag=f"g{nm}", name=f"g{nm}_{t}_{x}"
                        )
                        if t == 0:
                            nc.scalar.activation(g_t[:], xp_sb[:, m, :], fn)
                        else:
                            nc.vector.tensor_add(
                                out=g_t[:], in0=psums[m][:], in1=xp_sb[:, m, :]
                            )
                            nc.scalar.activation(g_t[:], g_t[:], fn)
                        pre[nm] = g_t

                    ig = work.tile([128, B], F32, tag="ig", name=f"ig_{t}_{x}")
                    nc.vector.tensor_mul(out=ig[:], in0=pre["i"][:], in1=pre["g"][:])
                    if t == 0:
                        nc.vector.tensor_copy(out=c_sb[:, x, :], in_=ig[:])
                    else:
                        fc_ = work.tile([128, B], F32, tag="fc", name=f"fc_{t}_{x}")
                        nc.vector.tensor_mul(
                            out=fc_[:], in0=pre["f"][:], in1=c_sb[:, x, :]
                        )
                        nc.vector.tensor_add(
                            out=c_sb[:, x, :], in0=ig[:], in1=fc_[:]
                        )
                    tc_t = work.tile([128, B], F32, tag="tc", name=f"tc_{t}_{x}")
                    nc.scalar.activation(tc_t[:], c_sb[:, x, :], AF.Tanh)
                    h_t = work.tile([128, B], DTR, tag=f"h{x}", name=f"h_{t}_{x}")
                    nc.vector.tensor_mul(out=h_t[:], in0=pre["o"][:], in1=tc_t[:])

                    # FC partial: outT[66, B] += wfc_x.T @ h_x
                    nc.tensor.matmul(
                        psfc,
                        wfc_sb[:, x, :],
                        h_t[:],
                        start=(x == 0),
                        stop=(x == 1),
                    )

                    if agin is not None:
                        nc.sync.dma_start(agin[x * 128 : (x + 1) * 128, :], h_t[:])

                if agin is not None:
                    agout = dramp.tile(
                        [NCORES * 256, B],
                        DTR,
                        tag="agout",
                        name=f"agout_{t}",
                        addr_space="Shared",
                    )
                    if variant == "full":
                        nc.gpsimd.collective_compute(
                            "AllGather",
                            mybir.AluOpType.bypass,
                            replica_groups=[list(range(NCORES))],
                            ins=[agin[:].opt()],
                            outs=[agout[:].opt()],
                            unique_tensors=os.environ.get("KERNEL_UT", "No"),
                        )
                    # split the gather-in so the first k-slots arrive (and the
                    # next step's matmuls can start) before the whole payload
                    # lands; leading chunks are smaller because the PE consumes
                    # slots in order and restarts on slot 0
                    ag_v = agout.rearrange("(s p) n -> p s n", p=128)
                    for lo, hi in ((0, 2), (2, 4), (4, 8), (8, 12), (12, 16)):
                        nc.sync.dma_start(
                            h_rhs[:, lo:hi, :],
                            ag_v[:, lo:hi, :],
                        )
                fc_stage = work.tile([OUTD, B], F32, tag="fcs", name=f"fcs_{t}")
                nc.scalar.copy(fc_stage[:], psfc[:])
                nc.sync.dma_start(out_ext[t], fc_stage[:])
                h_rhs_prev = h_rhs
    nc.finalize()
    return nc


def _prepare_inputs(inputs, labels, W_ih, W_hh, b_ih, b_hh, W_fc, b_fc):
    """Build per-core input maps. Returns (in_maps, frame0)."""
    inputs = np.asarray(inputs, dtype=np.float32)
    labels = np.asarray(labels)
    W_ih = np.asarray(W_ih, dtype=np.float32)
    W_hh = np.asarray(W_hh, dtype=np.float32)
    b_ih = np.asarray(b_ih, dtype=np.float32)
    b_hh = np.asarray(b_hh, dtype=np.float32)
    W_fc = np.asarray(W_fc, dtype=np.float32)
    b_fc = np.asarray(b_fc, dtype=np.float32)

    b = inputs.shape[0]
    frame0 = inputs.reshape(b, OUTD)
    enc = np.zeros((b, NCLS), dtype=np.float32)
    enc[:, int(labels[0])] = 1.0
    frame = np.concatenate([frame0, enc], axis=1)  # [B, 76]

    bias = b_ih + b_hh
    xproj = frame @ W_ih.T + bias  # [B, 8192]

    # global k-slot unit ordering: slot s<8: units 256*l + p (l=s); s>=8: 256*l+128+p
    in_maps = []
    for j in range(NCORES):
        rows = []
        for x in range(2):
            for q in range(4):
                base = q * H + HL * j + 128 * x
                rows.extend(range(base, base + 128))
        rows = np.array(rows)  # 1024 per-core gate rows

        Wj = W_hh[rows, :]  # [1024, 2048]
        # w[s, p, m] = Wj[m, unit(s,p)]; slot s = rank-major AG layout:
        # units [256*(s//2) + 128*(s%2), +128)
        w = np.empty((16, 128, 1024), dtype=np.float32)
        for s in range(16):
            l, x = s // 2, s % 2
            u0 = HL * l + 128 * x
            w[s] = Wj[:, u0 : u0 + 128].T
        xp = xproj[:, rows].T.reshape(8, 128, b).astype(np.float32)
        wfc = np.empty((2, 128, OUTD), dtype=np.float32)
        for x in range(2):
            u0 = HL * j + 128 * x
            wfc[x] = W_fc[:, u0 : u0 + 128].T
        if _MMDT == "bf16":
            import ml_dtypes

            w = w.astype(ml_dtypes.bfloat16)
            wfc = wfc.astype(ml_dtypes.bfloat16)
        in_maps.append({"w": w, "xp": np.ascontiguousarray(xp), "wfc": wfc})
    return in_maps, frame0, b_fc


_PROGRAM_CACHE = {}


def _get_program(T):
    variant = os.environ.get("KERNEL_VARIANT", "full")
    key = (T, variant)
    if key not in _PROGRAM_CACHE:
        _PROGRAM_CACHE[key] = _build_program(T, variant)
    return _PROGRAM_CACHE[key]


_RUNNER_CACHE = {}


def make_runner(nc):
    """Jitted runner for an arbitrary finalized program (mimics
    bass2jax.run_bass_via_pjrt's multi-core branch, but the traced callable is
    constructed ONCE so repeated calls hit jax's C++ fast path —
    run_bass_via_pjrt builds a fresh closure per call, which re-traces and
    re-serializes the whole BIR each time)."""
    import jax
    from jax.experimental.shard_map import shard_map
    from jax.sharding import Mesh, PartitionSpec

    from concourse import bass2jax, mybir as _mybir

    bass2jax.install_neuronx_cc_hook()
    partition_name = nc.partition_id_tensor.name if nc.partition_id_tensor else None
    in_names, out_names, out_avals, zero_outs = [], [], [], []
    for alloc in nc.m.functions[0].allocations:
        if not isinstance(alloc, _mybir.MemoryLocationSet):
            continue
        name = alloc.memorylocations[0].name
        if alloc.kind == "ExternalInput":
            if name != partition_name:
                in_names.append(name)
        elif alloc.kind == "ExternalOutput":
            shape = tuple(alloc.tensor_shape)
            dtype = _mybir.dt.np(alloc.dtype)
            out_names.append(name)
            out_avals.append(jax.core.ShapedArray(shape, dtype))
            zero_outs.append(np.zeros(shape, dtype))
    n_params = len(in_names)
    all_in_names = in_names + out_names
    if partition_name is not None:
        all_in_names.append(partition_name)

    def _body(*args):
        operands = list(args)
        if partition_name is not None:
            operands.append(bass2jax.partition_id_tensor())
        outs = bass2jax._bass_exec_p.bind(
            *operands,
            out_avals=tuple(out_avals),
            in_names=tuple(all_in_names),
            out_names=tuple(out_names),
            lowering_input_output_aliases=(),
            sim_require_finite=True,
            sim_require_nnan=True,
            nc=nc,
        )
        return tuple(outs)

    devices = jax.devices()[:NCORES]
    mesh = Mesh(np.asarray(devices), ("core",))
    n_outs = len(out_names)
    sharded = jax.jit(
        shard_map(
            _body,
            mesh=mesh,
            in_specs=(PartitionSpec("core"),) * (n_params + n_outs),
            out_specs=(PartitionSpec("core"),) * n_outs,
            check_rep=False,
        ),
        keep_unused=True,
    )
    meta = (in_names, out_names, out_avals, zero_outs, mesh)
    return (sharded, meta)


def _get_runner(T):
    key = (T, os.environ.get("KERNEL_VARIANT", "full"))
    if key not in _RUNNER_CACHE:
        _RUNNER_CACHE[key] = make_runner(_get_program(T))
    return _RUNNER_CACHE[key]


def run_fast(in_maps, T):
    """Run via the cached runner; returns list of per-core result dicts."""
    import jax

    sharded, (in_names, out_names, out_avals, zero_outs, mesh) = _get_runner(T)
    concat_in = [
        np.concatenate([in_maps[c][name] for c in range(NCORES)], axis=0)
        for name in in_names
    ]
    concat_zeros = [
        np.zeros((NCORES * z.shape[0], *z.shape[1:]), z.dtype) for z in zero_outs
    ]
    out_arrs = sharded(*concat_in, *concat_zeros)
    return [
        {
            name: np.asarray(out_arrs[i]).reshape(NCORES, *out_avals[i].shape)[c]
            for i, name in enumerate(out_names)
        }
        for c in range(NCORES)
    ]


def time_exec(in_maps, T, reps=8):
    """Time device execution with device-resident inputs (min over reps)."""
    import time as _time

    import jax

    sharded, (in_names, out_names, out_avals, zero_outs, mesh) = _get_runner(T)
    from jax.sharding import NamedSharding, PartitionSpec

    sh = NamedSharding(mesh, PartitionSpec("core"))
    dev_in = [
        jax.device_put(
            np.concatenate([in_maps[c][name] for c in range(NCORES)], axis=0), sh
        )
        for name in in_names
    ]
    dev_zero = [
        jax.device_put(np.zeros((NCORES * z.shape[0], *z.shape[1:]), z.dtype), sh)
        for z in zero_outs
    ]
    # warmup
    jax.block_until_ready(sharded(*dev_in, *dev_zero))
    best = float("inf")
    for _ in range(reps):
        t0 = _time.perf_counter()
        jax.block_until_ready(sharded(*dev_in, *dev_zero))
        best = min(best, _time.perf_counter() - t0)
    return best


def run(inputs_dict, T=LENGTH, trace=False):
    """Run the kernel; returns (full_output, exec_time_ns_or_None)."""
    in_maps, frame0, b_fc = _prepare_inputs(**inputs_dict)
    nc = _get_program(T)
    res = bass_utils.run_bass_kernel_spmd(
        nc, in_maps, core_ids=list(range(NCORES)), trace=trace
    )
    total = np.zeros((T, OUTD, B), dtype=np.float32)
    for r in res.results:
        total += r["outp"]
    total += b_fc[None, :, None]
    outs = total.transpose(2, 0, 1)  # [B, T, 66]
    full = np.concatenate([frame0[:, None, :], outs], axis=1)  # [B, T+1, 66]
    out = full.reshape(B, T + 1, 22, 3).astype(np.float32)
    return out, res.exec_time_ns


def kernel(**inputs):
    out, _ = run(inputs, T=LENGTH, trace=False)
    return out
